# revision 3
# baseline (speedup 1.0000x reference)
"""Trainium2 Bass kernel for nn_BilinearUpsampling_88373247082947.

Math (from the reference):
    outer[b,t,:] = (w[0] * x[b,t,:]) ⊗ x[b,t,:]              # (C*C,) row
    normed       = outer * rsqrt(max(sum(outer^2), EPS))
    out          = repeat(normed, 2, axis=1)                  # (B, 2T, C*C)

Key simplification: sum(outer^2) over the C*C axis equals (w^2) * (sum(x^2))^2,
so the normalizer is a per-frame scalar computed from ||x||^2 — the outer
product never needs to be materialized before normalization.

Per-frame output row:  out_row[c*C + d] = s_t * x[t,c] * x[t,d]
with s_t = w * rsqrt(max(w^2 * n_t^2, EPS)),  n_t = sum_c x[t,c]^2.

Sharding: pure data parallel over batch — core b handles example b
(B=8 == n_cores). Each core writes its own (2T, C*C) = 64 MiB slice; the
kernel is HBM-write-bound (~512 MiB total output).

Device layout (per core): frames on partitions. For each tile of 128 frames:
  - n = rowsum(x^2)  (DVE tensor_tensor_reduce)
  - s = w / sqrt(max(w^2 n^2, EPS))  (small [128,1] ops)
  - xs = x * s       (per-partition scalar multiply)
  - for c in range(128): ot[:, c*128:(c+1)*128] = x * xs[:, c]   (DVE 2x mode)
  - DMA ot twice to DRAM (even/odd output rows), 64 KiB contiguous/partition.
"""

import sys

import numpy as np

if "/opt/trn_rl_repo" not in sys.path:
    sys.path.insert(0, "/opt/trn_rl_repo")

B = 8
T = 512
C = 128
STRIDE = 2
EPS = 1e-12
N_CORES = 8
TT = 128          # frames per SBUF tile
NT = T // TT      # tiles per core
CC = C * C

_CACHE = {}


def _build_nc():
    """Build and compile the per-core Bass program (SPMD: same NEFF on all cores)."""
    from contextlib import ExitStack

    import concourse.bacc as bacc
    import concourse.mybir as mybir
    import concourse.tile as tile

    f32 = mybir.dt.float32
    Alu = mybir.AluOpType

    nc = bacc.Bacc("TRN2", target_bir_lowering=False, debug=False)

    x_d = nc.dram_tensor("x", [T, C], f32, kind="ExternalInput")
    w_d = nc.dram_tensor("w", [C], f32, kind="ExternalInput")  # host-replicated w[0]
    o_d = nc.dram_tensor("out", [T * STRIDE, CC], f32, kind="ExternalOutput")

    x_ap = x_d.ap()
    w_ap = w_d.ap()
    o_ap = o_d.ap()

    # out row index = 2*(i*TT + p) + r  ->  [i, p, r, d] view
    o_v = o_ap.rearrange("(i p r) d -> i p r d", p=TT, r=STRIDE)
    # x row index = i*TT + p  ->  [p, i, c] view (partition-major per tile)
    x_v = x_ap.rearrange("(i p) c -> p i c", p=TT)

    with tile.TileContext(nc) as tc, ExitStack() as ctx:
        const = ctx.enter_context(tc.tile_pool(name="const", bufs=1))
        small = ctx.enter_context(tc.tile_pool(name="small", bufs=2))
        outp = ctx.enter_context(tc.tile_pool(name="outp", bufs=2))

        x_all = const.tile([TT, NT, C], f32)
        nc.sync.dma_start(out=x_all[:, :, :], in_=x_v)

        w_bc = const.tile([TT, 1], f32)
        nc.sync.dma_start(out=w_bc[:, :], in_=w_ap.rearrange("(p c) -> p c", c=1))

        w2 = const.tile([TT, 1], f32)
        nc.vector.tensor_scalar(
            out=w2[:, :], in0=w_bc[:, :], scalar1=w_bc[:, 0:1], scalar2=None,
            op0=Alu.mult,
        )

        for i in range(NT):
            xt = x_all[:, i, :]  # [128, 128] current frame tile

            sq = small.tile([TT, C], f32, tag="sq")
            n = small.tile([TT, 1], f32, tag="n")
            nc.vector.tensor_tensor(out=sq[:, :], in0=xt, in1=xt, op=Alu.mult)
            nc.vector.reduce_sum(
                out=n[:, :], in_=sq[:, :], axis=mybir.AxisListType.X
            )
            # m = max(w^2 * n^2, EPS)
            m = small.tile([TT, 1], f32, tag="m")
            nc.vector.tensor_scalar(
                out=m[:, :], in0=n[:, :], scalar1=n[:, 0:1], scalar2=None,
                op0=Alu.mult,
            )
            nc.vector.tensor_scalar(
                out=m[:, :], in0=m[:, :], scalar1=w2[:, 0:1], scalar2=EPS,
                op0=Alu.mult, op1=Alu.max,
            )
            # s = w / sqrt(m)
            rt = small.tile([TT, 1], f32, tag="rt")
            nc.scalar.sqrt(out=rt[:, :], in_=m[:, :])
            inv = small.tile([TT, 1], f32, tag="inv")
            nc.vector.reciprocal(out=inv[:, :], in_=rt[:, :])
            s = small.tile([TT, 1], f32, tag="s")
            nc.vector.tensor_scalar(
                out=s[:, :], in0=inv[:, :], scalar1=w_bc[:, 0:1], scalar2=None,
                op0=Alu.mult,
            )
            # xs = x * s (per-partition scalar)
            xs = small.tile([TT, C], f32, tag="xs")
            nc.vector.tensor_scalar(
                out=xs[:, :], in0=xt, scalar1=s[:, 0:1], scalar2=None,
                op0=Alu.mult,
            )

            ot = outp.tile([TT, CC], f32)
            for c in range(C):
                nc.vector.tensor_scalar(
                    out=ot[:, c * C:(c + 1) * C], in0=xt, scalar1=xs[:, c:c + 1],
                    scalar2=None, op0=Alu.mult,
                )

            nc.sync.dma_start(out=o_v[i, :, 0, :], in_=ot[:, :])
            nc.sync.dma_start(out=o_v[i, :, 1, :], in_=ot[:, :])

    nc.compile()
    return nc


def _ensure_trace_support():
    """Install the NTFF profile hook that the image's antenv lacks.

    Only used by the dev/test harness (trace=True); the plain kernel() path
    never calls this.
    """
    import types

    import antenv

    if "antenv.axon_hooks" not in sys.modules:
        mod = types.ModuleType("antenv.axon_hooks")
        _state = {"hook": None}
        mod.set_axon_ntff_profile_hook = lambda h: _state.__setitem__("hook", h)
        mod.get_axon_ntff_profile_hook = lambda: _state["hook"]
        sys.modules["antenv.axon_hooks"] = mod
        antenv.axon_hooks = mod
    from antenv.axon_hooks import (
        get_axon_ntff_profile_hook,
        set_axon_ntff_profile_hook,
    )

    if get_axon_ntff_profile_hook() is None:
        from trn_agent_boot.trn_boot import _ntff_profile_via_ctypes

        set_axon_ntff_profile_hook(
            _ntff_profile_via_ctypes("/opt/axon/libaxon_pjrt.so")
        )
    import concourse.bass_utils as bu

    bu.upload_artifacts = lambda tmpdir: tmpdir


def _run(inputs, trace=False, **spmd_kwargs):
    """Shard, run on 8 cores, gather. Returns (full_output, BassKernelResults)."""
    from concourse.bass_utils import run_bass_kernel_spmd

    if trace:
        _ensure_trace_support()

    if "nc" not in _CACHE:
        _CACHE["nc"] = _build_nc()
    nc = _CACHE["nc"]

    x = np.ascontiguousarray(np.asarray(inputs["x"], dtype=np.float32))
    w = np.asarray(inputs["w"], dtype=np.float32).reshape(-1)
    assert x.shape == (B, T, C), x.shape
    w_rep = np.full((C,), w[0], dtype=np.float32)

    in_maps = [{"x": x[b], "w": w_rep} for b in range(N_CORES)]
    res = run_bass_kernel_spmd(
        nc, in_maps, core_ids=list(range(N_CORES)), trace=trace, **spmd_kwargs
    )
    out = np.stack([res.results[b]["out"] for b in range(N_CORES)], axis=0)
    return out, res


def kernel(**inputs) -> np.ndarray:
    out, _ = _run(inputs)
    return out
